# revision 46
# baseline (speedup 1.0000x reference)
"""Trainium2 Bass kernel for AngularMarginLoss (vocab-parallel softmax loss).

Problem: B=2048, D=256, C=100000, scale=30, margin=0.2, eps=1e-6.
  Wn = W / ||W||_row ; cos = clip(emb @ Wn.T, -1, 1)
  num_b = 30*cos(arccos(cos[b, t_b]) + 0.2)
  denom_b = exp(num_b) + sum_{c != t_b} exp(30*cos[b, c])
  loss = -mean(num_b - log(denom_b + 1e-6))

Sharding: tensor-parallel over the class dim C across 8 NeuronCores
(12500 classes/core, padded to 12544; classic vocab-parallel softmax).

v2 design (vs the v1 305us kernel):
  - W is pre-transposed PER SHARD on the host (pure layout change) to
    [d, c] so the fp8 DoubleRow matmul rhs comes straight from DRAM --
    no on-device PE transposes of W, no PSUM->SBUF copies for W, and
    the DMA moves 8KB contiguous lines.
  - W[targets] is gathered on the host (pure indexing); every core
    computes the identical f32 target-cosine locally, so the collective
    carries only the per-sample exp-sums (one 8KB AllReduce at the end).
  - Per-class normalization 30/||w_c||: DVE squares wT, tiny N=1
    matmuls (interleaved between main matmuls so their stationary loads
    hide) reduce over d into a compact [c_part, 16] PSUM tile, ScalarE
    rsqrt via Ln/Exp (same activation table set as the main Exp), PE
    transpose + K=1 outer-product matmuls broadcast it along the free
    axis, DVE multiplies + casts wT to fp8.
  - PSUM: two main pools of 4 and 3 banks (class groups alternate
    2048/1536 wide; last group 1792) + 1 bank for prep = 8 banks.
    16 j-tiles x 7 groups = 112 exp instructions (vs 144), cutting
    ScalarE's fixed 352-cycle/instruction overhead.
  - Final loss math is split: everything that only needs the target
    cosine runs mid-kernel; after the AllReduce only add+ln+sub+reduce
    remain (~2us tail).
The big matmul runs in fp8 DoubleRow (emb scaled x4, W side /4, so both
operands sit in fp8e4m3's healthy range); target-cosine path is f32.
Activation tables pinned to natural_log_exp_and_others so ScalarE never
reloads tables mid-kernel.
"""

import math
import sys

import numpy as np

if "/opt/trn_rl_repo" not in sys.path:
    sys.path.insert(0, "/opt/trn_rl_repo")

import concourse.bass as bass
import concourse.tile as tile
from concourse import bacc, mybir
from concourse.bass_utils import run_bass_kernel_spmd
from concourse.masks import make_identity

FP32 = mybir.dt.float32
BF16 = mybir.dt.bfloat16
FP8 = mybir.dt.float8e4
INT32 = mybir.dt.int32

N_CORES = 8
SCALE = 30.0
MARGIN = 0.2
EPS = 1e-6
D = 256  # embedding dim (2 partition tiles)
B = 2048
CSV = 12500  # valid classes per core
CPAD = 12544  # padded (98 * 128)
EMB_S = 4.0  # emb is scaled x4 into fp8; W side carries 30/4

_TABLES_PATCHED = False


def _patch_act_tables():
    """Force every activation fn we use into one table set so bacc never
    inserts mid-kernel ACT_TABLE_LOADs."""
    global _TABLES_PATCHED
    if _TABLES_PATCHED:
        return
    import functools

    import concourse.hw_specs as hw_specs

    orig = hw_specs.get_activation_tables
    KEEP = "natural_log_exp_and_others"
    A = mybir.ActivationFunctionType

    @functools.cache
    def patched(arch):
        tabs = {k: set(v) for k, v in orig(arch).items()}
        assert KEEP in tabs
        for name, fns in tabs.items():
            if name != KEEP:
                for f in (A.Exp, A.Ln, A.Copy, A.Identity):
                    fns.discard(f)
        return tabs

    hw_specs.get_activation_tables = patched
    bacc.get_activation_tables = patched
    _TABLES_PATCHED = True


# class-group sizes: row-tiles ping-pong between two 3-bank PSUM pools.
# graduated: group 0 small so the first exp starts early, group 1 midsized
# so group 1's prep chain still hides under group 0's exps.
SIZES = [640, 1152] + [1536] * 7
assert sum(SIZES) == CPAD
N_G = len(SIZES)
BASES = [sum(SIZES[:i]) for i in range(N_G)]


def build():
    n_bt = B // 128  # 16 b tiles
    ln30 = math.log(SCALE / EMB_S)  # folded: w8 = wT * (30/4)/||w||
    cos_m = math.cos(MARGIN)
    sin_m = math.sin(MARGIN)
    A = mybir.ActivationFunctionType
    O = mybir.AluOpType

    _patch_act_tables()
    nc = bacc.Bacc(
        "TRN2",
        target_bir_lowering=False,
        debug=False,
        num_devices=N_CORES,
    )

    emb_d = nc.declare_dram_parameter("emb", [B, D], FP32, isOutput=False)
    embt_d = nc.declare_dram_parameter("embT", [128, 2 * B], FP32, isOutput=False)
    w_d = nc.declare_dram_parameter("W", [128, 2 * CPAD], FP32, isOutput=False)
    tgt_d = nc.declare_dram_parameter("tgtw", [B, D], FP32, isOutput=False)
    out_d = nc.declare_dram_parameter("out", [1, 1], FP32, isOutput=True)

    # flat cc buffers: one contiguous descriptor per hop (a [128, 16]
    # layout becomes 128x 64B descriptors, each paced by a slow firmware
    # APB write -- ~40us; flat [1, 2048] moves as one 8KB transfer).
    # two phases so AR2's fixed ~11.5us trigger delay pipelines under AR1.
    cc1_in = nc.dram_tensor("cc1_in", [1, 128 * (B // 128)], FP32)
    cc1_out = nc.dram_tensor("cc1_out", [1, 128 * (B // 128)], FP32, addr_space="Shared")
    cc2_in = nc.dram_tensor("cc2_in", [1, 128 * (B // 128)], FP32)
    cc2_out = nc.dram_tensor("cc2_out", [1, 128 * (B // 128)], FP32, addr_space="Shared")

    with tile.TileContext(nc, num_cores=N_CORES) as tc:
        import contextlib

        with contextlib.ExitStack() as ctx:
            consts = ctx.enter_context(tc.tile_pool(name="consts", bufs=1))
            embf_p = ctx.enter_context(tc.tile_pool(name="embf", bufs=1))
            embt_p = ctx.enter_context(tc.tile_pool(name="embt", bufs=1))
            acc_p = ctx.enter_context(tc.tile_pool(name="acc", bufs=1))
            wf_p = ctx.enter_context(tc.tile_pool(name="wf", bufs=2))
            w8_p = ctx.enter_context(tc.tile_pool(name="w8", bufs=2))
            wsq_p = ctx.enter_context(tc.tile_pool(name="wsq", bufs=2))
            nrm_p = ctx.enter_context(tc.tile_pool(name="nrm", bufs=3))
            tgt_p = ctx.enter_context(tc.tile_pool(name="tgt", bufs=1))
            exp_p = ctx.enter_context(tc.tile_pool(name="expd", bufs=2))
            fin_p = ctx.enter_context(tc.tile_pool(name="fin", bufs=1))
            # PSUM: 3 + 3 + 2 banks
            ps_a = ctx.enter_context(tc.tile_pool(name="psa", bufs=1, space="PSUM"))
            ps_b = ctx.enter_context(tc.tile_pool(name="psb", bufs=1, space="PSUM"))
            ps_p = ctx.enter_context(tc.tile_pool(name="psp", bufs=2, space="PSUM"))

            # ---- constants ----
            ident = consts.tile([128, 128], BF16)
            make_identity(nc, ident[:])
            ones_c = consts.tile([128, 1], BF16)
            nc.vector.memset(ones_c[:], 1.0)
            ones8 = consts.tile([128, 32], FP8)
            nc.vector.memset(ones8[:], 1.0)
            ones83 = ones8[:].rearrange("p (two o) -> p two o", two=2)
            ones_k1 = consts.tile([1, 128], BF16)
            nc.vector.memset(ones_k1[:], 1.0)
            ones_f = consts.tile([128, 1], FP32)
            nc.vector.memset(ones_f[:], 1.0)
            b_tiny = consts.tile([128, 1], FP32)
            nc.vector.memset(b_tiny[:], 1e-30)
            b_ln30 = consts.tile([128, 1], FP32)
            nc.vector.memset(b_ln30[:], ln30)
            b_one = consts.tile([128, 1], FP32)
            nc.vector.memset(b_one[:], 1.0)
            b_lnssin = consts.tile([128, 1], FP32)
            nc.vector.memset(b_lnssin[:], math.log(SCALE * sin_m))
            b_eps = consts.tile([128, 1], FP32)
            nc.vector.memset(b_eps[:], EPS)

            # ---- embedding: host-pretransposed embT, just cast x4 -> fp8 ----
            embf = embf_p.tile([128, n_bt * D], FP32)  # [b_in_tile, j*D + d]
            embt = embt_p.tile([128, 2 * B], FP8)  # [d_in_tile, dt*B + j*128 + b]

            def emit_emb():
                embtf = embf_p.tile([128, 2 * B], FP32, name="embtf")
                for dt in range(2):
                    nc.sync.dma_start(
                        out=embtf[:, dt * B : (dt + 1) * B],
                        in_=embt_d[:, dt * B : (dt + 1) * B],
                    )
                for dt in range(2):
                    nc.vector.tensor_scalar(
                        out=embt[:, dt * B : (dt + 1) * B],
                        in0=embtf[:, dt * B : (dt + 1) * B],
                        scalar1=EMB_S, scalar2=None, op0=O.mult,
                    )

            def emit_embf_dma():
                # row-major emb, only needed by the target-cosine dots (g>=2)
                emb3o = embf[:].rearrange("p (j d) -> p j d", j=n_bt)
                emb3i = emb_d[:].rearrange("(j p) d -> p j d", p=128)
                for q in range(4):
                    j0, j1 = q * 4, q * 4 + 4
                    nc.sync.dma_start(out=emb3o[:, j0:j1], in_=emb3i[:, j0:j1])

            # ---- W prep for one group (prep ops returned as thunks) ----
            # state per group: wf (f32 wT), wsq (bf16), nrm psum, r30 bcast
            wstate: dict = {}

            def prep_dma(g):
                gw = SIZES[g]
                base = BASES[g]
                wf = wf_p.tile([128, 2 * 2048], FP32, tag="wf", name="wf")
                for dt in range(2):
                    nc.sync.dma_start(
                        out=wf[:, dt * 2048 : dt * 2048 + gw],
                        in_=w_d[:, dt * CPAD + base : dt * CPAD + base + gw],
                    )
                wstate[g] = {"wf": wf}

            def prep_sq(g):
                gw = SIZES[g]
                wf = wstate[g]["wf"]
                wsq = wsq_p.tile([128, 2 * 2048], FP8, tag="wsq", name="wsq")
                for dt in range(2):
                    nc.vector.tensor_tensor(
                        out=wsq[:, dt * 2048 : dt * 2048 + gw],
                        in0=wf[:, dt * 2048 : dt * 2048 + gw],
                        in1=wf[:, dt * 2048 : dt * 2048 + gw],
                        op=O.mult,
                    )
                wstate[g]["wsq"] = wsq

            def mini_thunks(g):
                """N=1 DoubleRow matmuls reducing wsq (fp8) over all 256 d
                in one pass -> nrm2 [c_part, n_ct]."""
                gw = SIZES[g]
                n_ct = gw // 128
                psn = ps_p.tile([128, 512], FP32, tag="psp", name="psn")
                wstate[g]["psn"] = psn
                wstate[g]["n_ct"] = n_ct
                thunks = []
                for t in range(n_ct):
                    def mk(t=t):
                        wsq = wstate[g]["wsq"]
                        wsq3 = wsq[:].rearrange("p (two c) -> p two c", two=2)
                        nc.tensor.matmul(
                            out=psn[:, t : t + 1],
                            lhsT=wsq3[:, :, t * 128 : (t + 1) * 128],
                            rhs=ones83[:, :, :1],
                            start=True,
                            stop=True,
                            perf_mode=mybir.MatmulPerfMode.DoubleRow,
                        )
                    thunks.append(mk)
                return thunks

            def prep_rsqrt(g):
                """compact rsqrt -> bf16 -> DMA-transpose to row [1, 2048]."""
                n_ct = wstate[g]["n_ct"]
                psn = wstate[g]["psn"]
                lng = nrm_p.tile([128, 16], FP32, tag="lng", name="lng")
                nc.scalar.activation(
                    lng[:, :n_ct], psn[:, :n_ct], A.Ln, bias=b_tiny[:, :1]
                )
                r30 = nrm_p.tile([128, 16], BF16, tag="r30", name="r30")
                # (30/4) / ||w|| = exp(-0.5*ln(nrm2) + ln(30/4))
                nc.scalar.activation(
                    r30[:, :n_ct], lng[:, :n_ct], A.Exp, scale=-0.5,
                    bias=b_ln30[:, :1],
                )
                # transpose to [n_ct, 128] (psum), then DMA into one row:
                # row[0, t*128 + p] = pst[t, p] = r30[p, t]
                pst = ps_p.tile([128, 512], BF16, tag="psp", name="pst")
                nc.tensor.transpose(
                    out=pst[:n_ct, :128], in_=r30[:, :n_ct], identity=ident[:]
                )
                rT = nrm_p.tile([16, 128], BF16, tag="rT", name="rT")
                nc.vector.tensor_copy(rT[:n_ct, :], pst[:n_ct, :128])
                row = nrm_p.tile([1, 2048], BF16, tag="row", name="row")
                nc.sync.dma_start(
                    out=row[:, : n_ct * 128], in_=rT[:n_ct, :]
                )
                wstate[g]["row"] = row

            def k1_scale_thunks(g):
                """Broadcast r30 along free axis (K=1 outer product) +
                scale+cast. Each batch: one K1 matmul covering 512 classes,
                then 2 DVE mults (dt=0,1) producing fp8 w8 slices."""
                gw = SIZES[g]
                n_ct = gw // 128
                w8 = w8_p.tile([128, 2 * 2048], FP8, tag="w8", name="w8")
                wstate[g]["w8"] = w8
                thunks = []
                n_batch = (n_ct + 3) // 4
                for sb in range(n_batch):
                    nts = min(4, n_ct - sb * 4)

                    def mk_k1(sb=sb, nts=nts):
                        row = wstate[g]["row"]
                        psk = ps_p.tile([128, 512], FP32, tag="psp", name="psk")
                        wstate[g]["psk"] = psk
                        nc.tensor.matmul(
                            out=psk[:, : nts * 128],
                            lhsT=ones_k1[:],
                            rhs=row[:, sb * 512 : sb * 512 + nts * 128],
                            start=True,
                            stop=True,
                        )
                    thunks.append(mk_k1)
                    for dt in range(2):
                        def mk_sc(sb=sb, nts=nts, dt=dt):
                            wf = wstate[g]["wf"]
                            psk = wstate[g]["psk"]
                            w8_ = wstate[g]["w8"]
                            c0 = sb * 512
                            nc.vector.tensor_tensor(
                                out=w8_[:, dt * 2048 + c0 : dt * 2048 + c0 + nts * 128],
                                in0=wf[:, dt * 2048 + c0 : dt * 2048 + c0 + nts * 128],
                                in1=psk[:, : nts * 128],
                                op=O.mult,
                            )
                        thunks.append(mk_sc)
                return thunks

            # ---- target path (host-gathered W[targets]) ----
            dots = acc_p.tile([128, n_bt], FP32)
            tnrm2 = acc_p.tile([128, n_bt], FP32)
            tgtf = tgt_p.tile([128, n_bt * D], FP32)

            def emit_tgt_dma():
                t3o = tgtf[:].rearrange("p (j d) -> p j d", j=n_bt)
                t3i = tgt_d[:].rearrange("(j p) d -> p j d", p=128)
                for q in range(4):
                    nc.sync.dma_start(out=t3o[:, q * 4 : q * 4 + 4], in_=t3i[:, q * 4 : q * 4 + 4])

            def tgt_thunks():
                thunks = []
                for j in range(n_bt):
                    def mk(j=j):
                        sc1 = wsq_p.tile([128, D], FP32, tag="tsc", name="sc1")
                        nc.vector.scalar_tensor_tensor(
                            out=sc1[:],
                            in0=embf[:, j * D : (j + 1) * D],
                            scalar=0.0,
                            in1=tgtf[:, j * D : (j + 1) * D],
                            op0=O.add,
                            op1=O.mult,
                            accum_out=dots[:, j : j + 1],
                        )
                    thunks.append(mk)
                for j in range(n_bt):
                    def mk(j=j):
                        sc2 = wsq_p.tile([128, D], FP32, tag="tsc", name="sc2")
                        nc.vector.scalar_tensor_tensor(
                            out=sc2[:],
                            in0=tgtf[:, j * D : (j + 1) * D],
                            scalar=0.0,
                            in1=tgtf[:, j * D : (j + 1) * D],
                            op0=O.add,
                            op1=O.mult,
                            accum_out=tnrm2[:, j : j + 1],
                        )
                    thunks.append(mk)
                return thunks

            # early final math: everything that only needs tcos
            early: dict = {}

            def emit_early_final():
                # tcos = dots * rsqrt(tnrm2); clip; num; e_n; e30t
                tln = fin_p.tile([128, n_bt], FP32, name="tln")
                nc.scalar.activation(tln[:], tnrm2[:], A.Ln, bias=b_tiny[:])
                trn = fin_p.tile([128, n_bt], FP32, name="trn")
                nc.scalar.activation(trn[:], tln[:], A.Exp, scale=-0.5)
                tc_ = fin_p.tile([128, n_bt], FP32, name="tc_")
                nc.vector.tensor_tensor(out=tc_[:], in0=dots[:], in1=trn[:], op=O.mult)
                xc = fin_p.tile([128, n_bt], FP32, name="xc")
                nc.vector.tensor_scalar(
                    out=xc[:], in0=tc_[:], scalar1=1.0, scalar2=-1.0,
                    op0=O.min, op1=O.max,
                )
                e30t = fin_p.tile([128, n_bt], FP32, name="e30t")
                nc.scalar.activation(e30t[:], xc[:], A.Exp, scale=SCALE)
                sq = fin_p.tile([128, n_bt], FP32, name="sq")
                nc.vector.tensor_tensor(out=sq[:], in0=xc[:], in1=xc[:], op=O.mult)
                lnu = fin_p.tile([128, n_bt], FP32, name="lnu")
                nc.scalar.activation(lnu[:], sq[:], A.Ln, scale=-1.0, bias=b_one[:])
                s30 = fin_p.tile([128, n_bt], FP32, name="s30")
                # 30*sin(m)*sqrt(1-sq) = exp(0.5*ln(1-sq) + ln(30*sin_m))
                nc.scalar.activation(s30[:], lnu[:], A.Exp, scale=0.5, bias=b_lnssin[:])
                num = fin_p.tile([128, n_bt], FP32, name="num")
                nc.vector.scalar_tensor_tensor(
                    out=num[:], in0=xc[:], scalar=SCALE * cos_m, in1=s30[:],
                    op0=O.mult, op1=O.subtract,
                )
                e_n = fin_p.tile([128, n_bt], FP32, name="e_n")
                nc.scalar.activation(e_n[:], num[:], A.Exp)
                # earlysum = e_n - e30t  (den = earlysum + global_sum)
                esum = fin_p.tile([128, n_bt], FP32, name="esum")
                nc.vector.tensor_tensor(out=esum[:], in0=e_n[:], in1=e30t[:], op=O.subtract)
                early["num"] = num
                early["esum"] = esum

            # ---- main loop ----
            # groups 0..N_G-2 accumulate into accs (AllReduce phase 1);
            # the last group into accs2 (small AllReduce phase 2 whose
            # trigger delay pipelines under phase 1's execution)
            accs = acc_p.tile([128, n_bt * (N_G - 1)], FP32)
            accs2 = acc_p.tile([128, n_bt], FP32)
            embt3 = embt[:].rearrange("p (two b) -> p two b", two=2)

            # startup: group-0/1 W DMA first (longer dependent chain), then
            # emb. Groups 0 AND 1 prep fully at startup -- group 0 is too
            # short (11us of exps) to host group 1's serial prep chain.
            prep_dma(0)
            prep_dma(1)
            emit_emb()
            for gg in (0, 1):
                prep_sq(gg)
                for th in mini_thunks(gg):
                    th()
                prep_rsqrt(gg)
                for th in k1_scale_thunks(gg):
                    th()

            tgt_list = tgt_thunks()
            s1 = fin_p.tile([128, n_bt], FP32, name="s1")

            def emit_cc1():
                for j in range(n_bt):
                    nc.vector.tensor_reduce(
                        out=s1[:, j : j + 1],
                        in_=accs[:, j * (N_G - 1) : (j + 1) * (N_G - 1)],
                        axis=mybir.AxisListType.X,
                        op=O.add,
                    )
                nc.sync.dma_start(out=cc1_in[:], in_=s1[:])
                nc.gpsimd.collective_compute(
                    "AllReduce",
                    O.add,
                    replica_groups=[list(range(N_CORES))],
                    ins=[cc1_in[:]],
                    outs=[cc1_out[:]],
                )

            for g in range(N_G):
                gw = SIZES[g]
                w8 = wstate[g]["w8"]
                w83 = w8[:].rearrange("p (two c) -> p two c", two=2)
                n_ch = (gw + 511) // 512
                gvalid = min(gw, CSV - BASES[g])  # exclude padded classes

                # Explicit per-j schedule for group g+1's prep:
                #   slot[(j,k)]: PE thunks emitted right after chunk-mm k of
                #   row-tile j (mini norm matmuls; their stationary loads
                #   hide under the preceding 512-wide main matmul).
                #   post[j]: ops emitted after row-tile j's exp (DVE work,
                #   scalar rsqrt chain, K1 broadcast batches).
                slot: dict = {}
                post: dict = {}
                if g + 2 < N_G:
                    # W DMA two groups ahead so it never gates g+1's prep
                    post.setdefault(8, []).append(lambda gg=g + 2: prep_dma(gg))
                if g + 1 < N_G and g >= 1:
                    post.setdefault(3, []).append(lambda gg=g + 1: prep_sq(gg))
                    minis = mini_thunks(g + 1)
                    # spread minis over chunk slots (j in [4, 10]) -- one
                    # per slot where possible so each mini's stationary
                    # load hides under the preceding 512-wide main matmul
                    per = -(-len(minis) // (7 * n_ch))
                    jj, kk = 4, 0
                    for i in range(0, len(minis), per):
                        assert jj <= 10, (g, jj)
                        slot.setdefault((jj, kk), []).extend(minis[i : i + per])
                        kk += 1
                        if kk >= n_ch:
                            kk = 0
                            jj += 1
                    post.setdefault(8, []).append(lambda gg=g + 1: prep_rsqrt(gg))
                    k1s = k1_scale_thunks(g + 1)
                    # each batch = [K1 matmul, dve mult dt0, dve mult dt1];
                    # early (posts 9..11) so w8 is ready well before the
                    # next group's first matmuls
                    for bi in range(0, len(k1s), 3):
                        post.setdefault(9 + bi // 3, []).extend(k1s[bi : bi + 3])
                if g == 0:
                    # bulk non-critical DMAs right after g2's W prefetch
                    post.setdefault(9, []).append(emit_tgt_dma)
                    post.setdefault(9, []).append(emit_embf_dma)
                if g == 2:
                    # 2 target-dot ops per row-tile (tgt DMA'd during g0/g1)
                    for j in range(n_bt):
                        post.setdefault(j, []).extend(tgt_list[2 * j : 2 * j + 2])
                if g == 3:
                    post.setdefault(6, []).append(emit_early_final)
                if g == N_G - 1:
                    # AllReduce phase 1 over groups 0..N_G-2: fires while
                    # the last group's exps still run
                    post.setdefault(0, []).append(emit_cc1)

                for j in range(n_bt):
                    # row-tiles ping-pong between the two 3-bank pools so
                    # exp(j) and matmul(j+1) never share a PSUM buffer
                    ps = (ps_a if j % 2 == 0 else ps_b).tile(
                        [128, 1536], FP32, tag="ps", name="ps",
                    )
                    for k in range(n_ch):
                        w0 = k * 512
                        w1 = min(gw, w0 + 512)
                        nc.tensor.matmul(
                            out=ps[:, w0:w1],
                            lhsT=embt3[:, :, j * 128 : (j + 1) * 128],
                            rhs=w83[:, :, w0:w1],
                            start=True,
                            stop=True,
                            perf_mode=mybir.MatmulPerfMode.DoubleRow,
                        )
                        for th in slot.get((j, k), ()):
                            th()
                    ed = exp_p.tile([128, 2048], BF16, tag="expd", name="ed")
                    acc_ap = (
                        accs[:, j * (N_G - 1) + g : j * (N_G - 1) + g + 1]
                        if g < N_G - 1
                        else accs2[:, j : j + 1]
                    )
                    nc.scalar.activation(
                        ed[:, :gvalid], ps[:, :gvalid], A.Exp, accum_out=acc_ap
                    )
                    for th in post.get(j, ()):
                        th()

            # ---- AllReduce phase 2 (last group's sums) ----
            nc.sync.dma_start(out=cc2_in[:], in_=accs2[:])
            nc.gpsimd.collective_compute(
                "AllReduce",
                O.add,
                replica_groups=[list(range(N_CORES))],
                ins=[cc2_in[:]],
                outs=[cc2_out[:]],
            )
            gs1 = fin_p.tile([128, n_bt], FP32)
            nc.sync.dma_start(out=gs1[:], in_=cc1_out[:])
            gs2 = fin_p.tile([128, n_bt], FP32)
            nc.sync.dma_start(out=gs2[:], in_=cc2_out[:])

            # ---- late final math: den = esum + gs1 + gs2; loss ----
            den0 = fin_p.tile([128, n_bt], FP32)
            nc.vector.tensor_tensor(out=den0[:], in0=early["esum"][:], in1=gs1[:], op=O.add)
            den = fin_p.tile([128, n_bt], FP32)
            nc.vector.tensor_tensor(out=den[:], in0=den0[:], in1=gs2[:], op=O.add)
            lden = fin_p.tile([128, n_bt], FP32)
            nc.scalar.activation(lden[:], den[:], A.Ln, bias=b_eps[:])
            pb = fin_p.tile([128, n_bt], FP32)
            red = fin_p.tile([128, 1], FP32)
            nc.vector.scalar_tensor_tensor(
                out=pb[:], in0=early["num"][:], scalar=0.0, in1=lden[:],
                op0=O.add, op1=O.subtract, accum_out=red[:],
            )
            psf = ps_p.tile([1, 1], FP32, tag="psp", name="psf")
            nc.tensor.matmul(out=psf[:], lhsT=red[:], rhs=ones_f[:], start=True, stop=True)
            res = fin_p.tile([1, 1], FP32)
            nc.vector.tensor_scalar(
                out=res[:], in0=psf[:], scalar1=-1.0 / B, scalar2=None, op0=O.mult
            )
            nc.sync.dma_start(out=out_d[:], in_=res[:])

    nc.compile()
    return nc


_CACHE: dict = {}


def _get(B_=B, csv=CSV):
    key = (B_, csv)
    if key not in _CACHE:
        _CACHE[key] = build()
    return _CACHE[key]


def make_in_maps(embedding, W, targets, B_=B, csv=CSV):
    emb = np.ascontiguousarray(embedding, dtype=np.float32)
    t64 = np.asarray(targets).astype(np.int64).reshape(-1)
    W = np.asarray(W, dtype=np.float32)
    tgtw = np.ascontiguousarray(W[t64])  # host gather: pure indexing
    # host transpose (pure layout): embT[p, dt*B + b] = emb[b, dt*128+p]
    embT = np.ascontiguousarray(
        emb.T.reshape(2, 128, B).transpose(1, 0, 2).reshape(128, 2 * B)
    )
    in_maps = []
    for i in range(N_CORES):
        c0 = i * csv
        # host transpose (pure layout): wT[p, dt*CPAD + c] = W[c0+c, dt*128+p]
        wt = np.zeros((128, 2 * CPAD), dtype=np.float32)
        blk = W[c0 : c0 + csv].T.reshape(2, 128, csv)  # [dt, p, c]
        wt[:, 0:csv] = blk[0]
        wt[:, CPAD : CPAD + csv] = blk[1]
        in_maps.append({"emb": emb, "embT": embT, "W": wt, "tgtw": tgtw})
    return in_maps


def kernel(embedding, W, targets):
    assert embedding.shape == (B, D) and W.shape == (N_CORES * CSV, D)
    nc = _get()
    in_maps = make_in_maps(embedding, W, targets)
    res = run_bass_kernel_spmd(nc, in_maps, list(range(N_CORES)))
    return np.asarray(res.results[0]["out"][0, 0], dtype=np.float32)
